# revision 13
# baseline (speedup 1.0000x reference)
"""BCH/RS systematic encoder kernel for Trainium2 (8 NeuronCores, data parallel).

Computes out = concat([msg, (msg @ Gp) mod 2], axis=-1) for
msg [16384, 1000] f32 of 0/1 bits and Gp [1000, 256] f32 of 0/1 bits.

Design v16 (per core, 2048 rows = 4 chunks of 512, parity-only device):
  - Host assembles the systematic half of the codeword from the input;
    the device only computes the parity block (removes the 10.3 MB/core
    f32 output write). Per-core HBM traffic: 2.36 MB fp8 read + 1.05 MB
    i16 write.
  - Host ships msg as fp8e4 (exact 0/1) pre-transposed to the matmul
    moving layout msgt[q, c, g, i, m] = msg[512c + m, 256g + 128i + q];
    Gp blocks are the stationary DoubleRow operand, 512 rows stream per
    matmul into [128, 512] f32 PSUM holding parity transposed.
  - The PE pstate ramps 1.2 -> 2.4 GHz only after ~9 us of sustained
    activity (measured: 427 ns -> 216 ns per 512-stream matmul), so four
    dummy warmup matmuls on zeroed scratch start the ramp clock during
    the load prologue.
  - HWDGE "dynamic" queues generate descriptors ON the issuing sequencer
    (~0.6 us DIRECT2D per batch, ~2 ns/descriptor): consolidated loads
    (chunk 0 split per g for a fast first matmul, whole-chunk pieces
    after) keep generation ahead of the wire; everything rides the sync
    ring. ACT is completely idle (scalar.copy would stall the context
    entry barrier ~1.3 us on ACT_TABLE_LOAD).
  - Mod 2 through integers (TS bitvec ops can't cast, AluOp.mod invalid):
    DVE evicts PSUM f32 -> i16 (exact, sums <= 1000) and ANDs with 1 at
    the 2x 16-bit rate, one whole-chunk [128, 2, 512] op pair per chunk
    (PSUM tiles span 2 banks; matmuls never cross a bank). The LAST
    chunk evicts per 512-row half to shorten the post-PE serial chain.
    i16 parity stores ride the sync ring behind the loads; host upcasts.
  - Host gathers: upcast i16 parity -> f32, un-transpose, concatenate
    with the original f32 message bits.
"""

import os
import sys

import numpy as np

if os.path.isdir("/opt/trn_rl_repo") and "/opt/trn_rl_repo" not in sys.path:
    sys.path.insert(0, "/opt/trn_rl_repo")

import ml_dtypes

import concourse.bacc as bacc
import concourse.mybir as mybir
import concourse.tile as tile
from concourse.bass_utils import run_bass_kernel_spmd

BATCH = 16384
MSG = 1000
NPAR = 256
NCORES = 8
ROWS = BATCH // NCORES  # 2048
P = 128
KB = 4  # k pair-blocks of 256; padded K = 1024
KPAD = KB * 2 * P
CH = 4 * P  # rows streamed per matmul (one PSUM bank of f32)

# test.py pokes these for profiling
TRACE = False
LAST_RESULT = None

_CACHE = {}

F8 = mybir.dt.float8e4
I16 = mybir.dt.int16
F32 = mybir.dt.float32


def build_nc(rows=ROWS):
    """Emit the Bass/Tile IR for one core handling `rows` rows."""
    n_chunks = rows // CH
    nc = bacc.Bacc("TRN2", target_bir_lowering=False, debug=False)
    msgt = nc.dram_tensor(
        "msgt", [P, n_chunks, KB, 2, CH], F8, kind="ExternalInput"
    )
    gp = nc.dram_tensor("gp", [P, KB, 2, NPAR], F8, kind="ExternalInput")
    out = nc.dram_tensor(
        "out", [P, n_chunks, 2, CH], I16, kind="ExternalOutput"
    )

    with tile.TileContext(nc) as tc:
        with (
            tc.tile_pool(name="gpool", bufs=1) as gpool,
            tc.tile_pool(name="cpool", bufs=2) as cpool,
            tc.tile_pool(name="opool", bufs=1) as opool,
            tc.tile_pool(name="ppool", bufs=2, space="PSUM") as ppool,
            tc.tile_pool(name="wpool", bufs=1, space="PSUM") as wpool,
        ):
            gp2 = gp[:, :, :, :].rearrange("q g i n -> q (g i n)")
            msgt2 = msgt[:, :, :, :, :].rearrange("q c g i m -> q (c g i m)")
            out2 = out[:, :, :, :].rearrange("q c h m -> q (c h m)")

            # PE pstate warmup: matmuls on zeroed scratch (the ramp clock
            # needs ~7.5 us of cumulative PE busy; results never read).
            # Memsets ride DVE, which is otherwise idle until ~13 us.
            wW = gpool.tile([P, P], F8, tag="wW")
            nc.vector.memset(wW[:, :], 0)
            wX = gpool.tile([P, CH], F8, tag="wX")
            nc.vector.memset(wX[:, :], 0)
            wacc = wpool.tile([P, CH], F32, tag="warm")
            for _ in range(6):
                nc.tensor.matmul(
                    wacc[:, :], wW[:, :], wX[:, :], start=True, stop=True
                )

            # Descriptor generation (~0.65 us per dma_start, serial per
            # sequencer) is split across BOTH HWDGE rings: sync takes the
            # front of the pipeline in consumption order, scalar takes the
            # back chunks + all parity stores.
            piece = 2 * CH
            gsb0 = gpool.tile([P, 2 * NPAR], F8, tag="g0")
            nc.sync.dma_start(out=gsb0[:, :], in_=gp2[:, 0 : 2 * NPAR])
            m0a = gpool.tile([P, 2 * piece], F8, tag="m0a")
            nc.sync.dma_start(out=m0a[:, :], in_=msgt2[:, 0 : 2 * piece])
            gsb123 = gpool.tile([P, 3 * 2 * NPAR], F8, tag="g123")
            nc.sync.dma_start(out=gsb123[:, :], in_=gp2[:, 2 * NPAR :])
            m0b = gpool.tile([P, 2 * piece], F8, tag="m0b")
            nc.sync.dma_start(
                out=m0b[:, :], in_=msgt2[:, 2 * piece : 4 * piece]
            )
            ctiles = {}
            for c in range(1, n_chunks):
                t = gpool.tile([P, KB * piece], F8, tag=f"c{c}")
                ring = nc.sync if c == 1 else nc.scalar
                ring.dma_start(
                    out=t[:, :],
                    in_=msgt2[:, c * KB * piece : (c + 1) * KB * piece],
                )
                ctiles[c] = t

            def gsbv(g):
                if g == 0:
                    return gsb0[:, :].rearrange("q (i n) -> q i n", n=NPAR)
                s = gsb123[:, (g - 1) * 2 * NPAR : g * 2 * NPAR]
                return s.rearrange("q (i n) -> q i n", n=NPAR)

            def mtv(c, g):
                if c == 0:
                    t = m0a if g < 2 else m0b
                    s = t[:, (g % 2) * piece : (g % 2 + 1) * piece]
                else:
                    s = ctiles[c][:, g * piece : (g + 1) * piece]
                return s.rearrange("q (i m) -> q i m", m=CH)

            for c in range(n_chunks):
                # acc[nh, h, m] = sum_k msg[512c + m, k] Gp[k, 128h + nh]
                acc = ppool.tile([P, 2, CH], F32, tag="acc")
                for h in range(2):
                    for g in range(KB):
                        nc.tensor.matmul(
                            acc[:, h, :],
                            gsbv(g)[:, :, h * P : (h + 1) * P],
                            mtv(c, g)[:, :, :],
                            start=(g == 0),
                            stop=(g == KB - 1),
                            perf_mode=mybir.MatmulPerfMode.DoubleRow,
                        )
                if c < n_chunks - 1:
                    # whole-chunk evict: one DVE op pair over [128, 2, 512]
                    ci = cpool.tile([P, 2, CH], I16, tag="ci")
                    nc.vector.tensor_copy(ci[:, :, :], acc[:, :, :])
                    e = opool.tile([P, 2, CH], I16, tag=f"e{c}")
                    nc.vector.tensor_scalar(
                        e[:, :, :],
                        ci[:, :, :],
                        1,
                        None,
                        mybir.AluOpType.bitwise_and,
                    )
                    nc.scalar.dma_start(
                        out=out2[:, c * 2 * CH : (c + 1) * 2 * CH],
                        in_=e[:, :, :].rearrange("q h m -> q (h m)"),
                    )
                else:
                    # last chunk per half: shorter post-PE serial chain
                    for h in range(2):
                        ci = cpool.tile([P, CH], I16, tag="cil")
                        nc.vector.tensor_copy(ci[:, :], acc[:, h, :])
                        e = opool.tile([P, CH], I16, tag=f"el{h}")
                        nc.vector.tensor_scalar(
                            e[:, :], ci[:, :], 1, None,
                            mybir.AluOpType.bitwise_and,
                        )
                        nc.scalar.dma_start(
                            out=out2[
                                :,
                                (2 * c + h) * CH : (2 * c + h + 1) * CH,
                            ],
                            in_=e[:, :],
                        )

    nc.compile()
    return nc


def prep_gp(Gp):
    """Pad Gp to 1024 rows and swizzle to [128, 4, 2, 256] fp8:
    gsw[q, g, i, n] = Gp_pad[256*g + 128*i + q, n]
    """
    gp = np.asarray(Gp, dtype=np.float32)
    gp_pad = np.zeros((KPAD, NPAR), dtype=np.float32)
    gp_pad[:MSG] = gp
    gsw = gp_pad.reshape(KB, 2, P, NPAR).transpose(2, 0, 1, 3)
    return np.ascontiguousarray(gsw).astype(ml_dtypes.float8_e4m3)


def prep_msgt(msg, rows=ROWS):
    """Cast 0/1 f32 message bits to fp8 (exact), pad k to 1024, and swizzle
    each `rows`-row slice to the transposed moving layout
    msgt[q, c, g, i, m] = msg[slice_row0 + 512c + m, 256g + 128i + q]."""
    f8 = np.zeros((msg.shape[0], KPAD), dtype=ml_dtypes.float8_e4m3)
    f8[:, :MSG] = msg.astype(ml_dtypes.float8_e4m3)
    n_chunks = rows // CH
    per_core = []
    for i in range(msg.shape[0] // rows):
        sl = f8[i * rows : (i + 1) * rows]
        # [c, m, g, i, q] -> [q, c, g, i, m]
        sw = sl.reshape(n_chunks, CH, KB, 2, P).transpose(4, 0, 2, 3, 1)
        per_core.append(np.ascontiguousarray(sw))
    return per_core


def parity_from_out(out_i16):
    """Device 'out' [128, n_chunks, 2, CH] i16 -> [rows, 256] f32."""
    o = np.asarray(out_i16)
    n_chunks = o.shape[1]
    # [nh, c, h, m] -> [c, m, h, nh] -> [rows, 256]
    return (
        o.transpose(1, 3, 2, 0)
        .reshape(n_chunks * CH, NPAR)
        .astype(np.float32)
    )


def kernel(message_bits, Gp):
    global LAST_RESULT
    msg = np.ascontiguousarray(np.asarray(message_bits, dtype=np.float32))
    assert msg.shape == (BATCH, MSG), msg.shape
    gsw = prep_gp(Gp)
    msg_cores = prep_msgt(msg)

    if "nc" not in _CACHE:
        _CACHE["nc"] = build_nc()
    nc = _CACHE["nc"]

    in_maps = [{"msgt": msg_cores[i], "gp": gsw} for i in range(NCORES)]
    res = run_bass_kernel_spmd(
        nc, in_maps, core_ids=list(range(NCORES)), trace=TRACE
    )
    LAST_RESULT = res

    full = np.empty((BATCH, MSG + NPAR), dtype=np.float32)
    full[:, :MSG] = msg
    for i, r in enumerate(res.results):
        full[i * ROWS : (i + 1) * ROWS, MSG:] = parity_from_out(r["out"])
    return full


# revision 15
# speedup vs baseline: 1.1060x; 1.1060x over previous
"""BCH/RS systematic encoder kernel for Trainium2 (8 NeuronCores, data parallel).

Computes out = concat([msg, (msg @ Gp) mod 2], axis=-1) for
msg [16384, 1000] f32 of 0/1 bits and Gp [1000, 256] f32 of 0/1 bits.

Design v16 (per core, 2048 rows = 4 chunks of 512, parity-only device):
  - Host assembles the systematic half of the codeword from the input;
    the device only computes the parity block (removes the 10.3 MB/core
    f32 output write). Per-core HBM traffic: 2.36 MB fp8 read + 1.05 MB
    i16 write.
  - Host ships msg as fp8e4 (exact 0/1) pre-transposed to the matmul
    moving layout msgt[q, c, g, i, m] = msg[512c + m, 256g + 128i + q];
    Gp blocks are the stationary DoubleRow operand, 512 rows stream per
    matmul into [128, 512] f32 PSUM holding parity transposed.
  - The PE pstate ramps 1.2 -> 2.4 GHz only after ~9 us of sustained
    activity (measured: 427 ns -> 216 ns per 512-stream matmul), so four
    dummy warmup matmuls on zeroed scratch start the ramp clock during
    the load prologue.
  - HWDGE "dynamic" queues generate descriptors ON the issuing sequencer
    (~0.6 us DIRECT2D per batch, ~2 ns/descriptor): consolidated loads
    (chunk 0 split per g for a fast first matmul, whole-chunk pieces
    after) keep generation ahead of the wire; everything rides the sync
    ring. ACT is completely idle (scalar.copy would stall the context
    entry barrier ~1.3 us on ACT_TABLE_LOAD).
  - Mod 2 through integers (TS bitvec ops can't cast, AluOp.mod invalid):
    DVE evicts PSUM f32 -> i16 (exact, sums <= 1000) and ANDs with 1 at
    the 2x 16-bit rate, one whole-chunk [128, 2, 512] op pair per chunk
    (PSUM tiles span 2 banks; matmuls never cross a bank). The LAST
    chunk evicts per 512-row half to shorten the post-PE serial chain.
    i16 parity stores ride the sync ring behind the loads; host upcasts.
  - Host gathers: upcast i16 parity -> f32, un-transpose, concatenate
    with the original f32 message bits.
"""

import os
import sys

import numpy as np

if os.path.isdir("/opt/trn_rl_repo") and "/opt/trn_rl_repo" not in sys.path:
    sys.path.insert(0, "/opt/trn_rl_repo")

import ml_dtypes

import concourse.bacc as bacc
import concourse.mybir as mybir
import concourse.tile as tile
from concourse.bass_utils import run_bass_kernel_spmd

BATCH = 16384
MSG = 1000
NPAR = 256
NCORES = 8
ROWS = BATCH // NCORES  # 2048
P = 128
KB = 4  # k pair-blocks of 256; padded K = 1024
KPAD = KB * 2 * P
CH = 4 * P  # rows streamed per matmul (one PSUM bank of f32)

# test.py pokes these for profiling
TRACE = False
LAST_RESULT = None

_CACHE = {}

F8 = mybir.dt.float8e4
I16 = mybir.dt.int16
F32 = mybir.dt.float32


def build_nc(rows=ROWS):
    """Emit the Bass/Tile IR for one core handling `rows` rows."""
    n_chunks = rows // CH
    nc = bacc.Bacc("TRN2", target_bir_lowering=False, debug=False)
    msgt = nc.dram_tensor(
        "msgt", [P, n_chunks, KB, 2, CH], F8, kind="ExternalInput"
    )
    gp = nc.dram_tensor("gp", [P, KB, 2, NPAR], F8, kind="ExternalInput")
    out = nc.dram_tensor(
        "out", [P, n_chunks, 2, CH], I16, kind="ExternalOutput"
    )

    with tile.TileContext(nc) as tc:
        with (
            tc.tile_pool(name="gpool", bufs=1) as gpool,
            tc.tile_pool(name="cpool", bufs=2) as cpool,
            tc.tile_pool(name="opool", bufs=1) as opool,
            tc.tile_pool(name="ppool", bufs=2, space="PSUM") as ppool,
            tc.tile_pool(name="wpool", bufs=1, space="PSUM") as wpool,
        ):
            gp2 = gp[:, :, :, :].rearrange("q g i n -> q (g i n)")
            msgt2 = msgt[:, :, :, :, :].rearrange("q c g i m -> q (c g i m)")
            out2 = out[:, :, :, :].rearrange("q c h m -> q (c h m)")

            # PE pstate warmup: short matmuls on zeroed scratch (the ramp
            # clock needs ~7.5 us of cumulative PE busy; results never
            # read). One tiny DVE memset so warm MM0 isn't gated on a
            # slower engine's context entry.
            wW = gpool.tile([P, P], F8, tag="wW")
            nc.vector.memset(wW[:, :], 0)
            wacc = wpool.tile([P, P], F32, tag="warm")
            for _ in range(16):
                nc.tensor.matmul(
                    wacc[:, :], wW[:, :], wW[:, :], start=True, stop=True
                )

            # Descriptor generation (~0.65 us per dma_start, serial per
            # sequencer) is split across BOTH HWDGE rings: sync takes the
            # front of the pipeline in consumption order, scalar takes the
            # back chunks + all parity stores.
            piece = 2 * CH
            gsb0 = gpool.tile([P, 2 * NPAR], F8, tag="g0")
            nc.sync.dma_start(out=gsb0[:, :], in_=gp2[:, 0 : 2 * NPAR])
            m0a = gpool.tile([P, 2 * piece], F8, tag="m0a")
            nc.sync.dma_start(out=m0a[:, :], in_=msgt2[:, 0 : 2 * piece])
            gsb123 = gpool.tile([P, 3 * 2 * NPAR], F8, tag="g123")
            nc.sync.dma_start(out=gsb123[:, :], in_=gp2[:, 2 * NPAR :])
            m0b = gpool.tile([P, 2 * piece], F8, tag="m0b")
            nc.sync.dma_start(
                out=m0b[:, :], in_=msgt2[:, 2 * piece : 4 * piece]
            )
            ctiles = {}
            for c in range(1, n_chunks):
                t = gpool.tile([P, KB * piece], F8, tag=f"c{c}")
                nc.sync.dma_start(
                    out=t[:, :],
                    in_=msgt2[:, c * KB * piece : (c + 1) * KB * piece],
                )
                ctiles[c] = t

            def gsbv(g):
                if g == 0:
                    return gsb0[:, :].rearrange("q (i n) -> q i n", n=NPAR)
                s = gsb123[:, (g - 1) * 2 * NPAR : g * 2 * NPAR]
                return s.rearrange("q (i n) -> q i n", n=NPAR)

            def mtv(c, g):
                if c == 0:
                    t = m0a if g < 2 else m0b
                    s = t[:, (g % 2) * piece : (g % 2 + 1) * piece]
                else:
                    s = ctiles[c][:, g * piece : (g + 1) * piece]
                return s.rearrange("q (i m) -> q i m", m=CH)

            for c in range(n_chunks):
                # acc[nh, h, m] = sum_k msg[512c + m, k] Gp[k, 128h + nh]
                acc = ppool.tile([P, 2, CH], F32, tag="acc")
                for h in range(2):
                    for g in range(KB):
                        nc.tensor.matmul(
                            acc[:, h, :],
                            gsbv(g)[:, :, h * P : (h + 1) * P],
                            mtv(c, g)[:, :, :],
                            start=(g == 0),
                            stop=(g == KB - 1),
                            perf_mode=mybir.MatmulPerfMode.DoubleRow,
                        )
                if c < n_chunks - 1:
                    # whole-chunk evict: one DVE op pair over [128, 2, 512]
                    ci = cpool.tile([P, 2, CH], I16, tag="ci")
                    nc.vector.tensor_copy(ci[:, :, :], acc[:, :, :])
                    e = opool.tile([P, 2, CH], I16, tag=f"e{c}")
                    nc.vector.tensor_scalar(
                        e[:, :, :],
                        ci[:, :, :],
                        1,
                        None,
                        mybir.AluOpType.bitwise_and,
                    )
                    nc.scalar.dma_start(
                        out=out2[:, c * 2 * CH : (c + 1) * 2 * CH],
                        in_=e[:, :, :].rearrange("q h m -> q (h m)"),
                    )
                else:
                    # last chunk per half: shorter post-PE serial chain
                    for h in range(2):
                        ci = cpool.tile([P, CH], I16, tag="cil")
                        nc.vector.tensor_copy(ci[:, :], acc[:, h, :])
                        e = opool.tile([P, CH], I16, tag=f"el{h}")
                        nc.vector.tensor_scalar(
                            e[:, :], ci[:, :], 1, None,
                            mybir.AluOpType.bitwise_and,
                        )
                        nc.scalar.dma_start(
                            out=out2[
                                :,
                                (2 * c + h) * CH : (2 * c + h + 1) * CH,
                            ],
                            in_=e[:, :],
                        )

    nc.compile()
    return nc


def prep_gp(Gp):
    """Pad Gp to 1024 rows and swizzle to [128, 4, 2, 256] fp8:
    gsw[q, g, i, n] = Gp_pad[256*g + 128*i + q, n]
    """
    gp = np.asarray(Gp, dtype=np.float32)
    gp_pad = np.zeros((KPAD, NPAR), dtype=np.float32)
    gp_pad[:MSG] = gp
    gsw = gp_pad.reshape(KB, 2, P, NPAR).transpose(2, 0, 1, 3)
    return np.ascontiguousarray(gsw).astype(ml_dtypes.float8_e4m3)


def prep_msgt(msg, rows=ROWS):
    """Cast 0/1 f32 message bits to fp8 (exact), pad k to 1024, and swizzle
    each `rows`-row slice to the transposed moving layout
    msgt[q, c, g, i, m] = msg[slice_row0 + 512c + m, 256g + 128i + q]."""
    f8 = np.zeros((msg.shape[0], KPAD), dtype=ml_dtypes.float8_e4m3)
    f8[:, :MSG] = msg.astype(ml_dtypes.float8_e4m3)
    n_chunks = rows // CH
    per_core = []
    for i in range(msg.shape[0] // rows):
        sl = f8[i * rows : (i + 1) * rows]
        # [c, m, g, i, q] -> [q, c, g, i, m]
        sw = sl.reshape(n_chunks, CH, KB, 2, P).transpose(4, 0, 2, 3, 1)
        per_core.append(np.ascontiguousarray(sw))
    return per_core


def parity_from_out(out_i16):
    """Device 'out' [128, n_chunks, 2, CH] i16 -> [rows, 256] f32."""
    o = np.asarray(out_i16)
    n_chunks = o.shape[1]
    # [nh, c, h, m] -> [c, m, h, nh] -> [rows, 256]
    return (
        o.transpose(1, 3, 2, 0)
        .reshape(n_chunks * CH, NPAR)
        .astype(np.float32)
    )


def kernel(message_bits, Gp):
    global LAST_RESULT
    msg = np.ascontiguousarray(np.asarray(message_bits, dtype=np.float32))
    assert msg.shape == (BATCH, MSG), msg.shape
    gsw = prep_gp(Gp)
    msg_cores = prep_msgt(msg)

    if "nc" not in _CACHE:
        _CACHE["nc"] = build_nc()
    nc = _CACHE["nc"]

    in_maps = [{"msgt": msg_cores[i], "gp": gsw} for i in range(NCORES)]
    res = run_bass_kernel_spmd(
        nc, in_maps, core_ids=list(range(NCORES)), trace=TRACE
    )
    LAST_RESULT = res

    full = np.empty((BATCH, MSG + NPAR), dtype=np.float32)
    full[:, :MSG] = msg
    for i, r in enumerate(res.results):
        full[i * ROWS : (i + 1) * ROWS, MSG:] = parity_from_out(r["out"])
    return full
